# revision 27
# baseline (speedup 1.0000x reference)
"""Trainium2 Bass kernel for nn_AllToAllMoE (degenerate single-group MoE == dense MLP).

reference:  y = gelu(x @ w1 + b1, exact) @ w2 + b2
  x  (16384, 2048) f32
  w1 (2048, 8192) f32, b1 (8192,) f32
  w2 (8192, 2048) f32, b2 (2048,) f32

Strategy: the all_to_all with a single replica group is an identity permutation,
so the problem is a dense 2-layer MLP. TOKENS are sharded across the 8
NeuronCores (data parallel, no collectives). GEMM1 is computed in the
transposed orientation (ffn dim on partitions, tokens on the free dim);
GEMM2 is computed in the natural orientation (tokens on partitions, hidden
on the free dim) so its products run at FD=512 and the output DMAs out
row-major [T, H] with no host transpose.

GEMM1 (x @ w1) uses one level of Strassen per 512-token block:
  tokens 512 -> halves 256, K 2048 -> halves 1024, F 8192 -> halves 4096.
  7 products P_i = Ac_i @ Bc_i accumulate in PSUM (FD=256, two products
  packed per 2KB PSUM bank). A-side combos (of x) are precomputed on the
  host and shipped as `xc`; B-side combos (of w1) are built on-chip by the
  Vector engine from a column-pair slab of the permuted `w1p`; products are
  drained PSUM->SBUF (bf16) by the Scalar engine; the C-quadrant combines
  run on Vector+GpSimd(Pool) in bf16; ScalarE applies exact GELU + b1.

GEMM2 (h @ w2) uses one level of Strassen as well, in transposed-output
form:  y[T, H] = h^T[T, F] @ w2[F, H], lhsT = hT pieces (A-side combos of
h quadrants built on the fly by the Vector engine from hT), rhs = w2
quadrant combos precomputed on the HOST and shipped as `w2c` (7/4 x w2
bytes, streamed per block). Products accumulate in 4 PSUM banks (FD=512,
so the PE - not LDWEIGHTS or DMA - is the pacer), are drained with fused
+/- adds into a bf16 accumulator Cacc[T-part, H], and each output
quadrant DMAs out (bf16, host converts to f32) as soon as its last
product lands. b2 is injected via K=1 matmuls (ones (x) b2row) into the
PSUM init of products P2 and P3 (which together touch each output
quadrant exactly once, always with + sign).

Perf-critical details learned on HW (do not regress):
 - Every streamed tensor (w1p slabs, w2c chunks, xc blocks) is packed
   host-side so each DMA transfer is fully contiguous in DRAM; strided
   2KB-line layouts cap at ~240GB/s vs ~300+ contiguous, which starves
   phase B (w2c needs ~294GB/s sustained to keep the PE fed).
 - Next block's xc/slab prologue DMAs are spread between phase-B
   products; issued all-at-once after phase B they queue behind 58MB of
   w2c and arrive after phase A has started (~12us/block of PE stalls).
 - slab_bufs=3: with 2, the slab(j+2) DMA start serializes behind slab(j)
   reads and w1 combos go late (~60us/block). wc combos must stay on the
   Vector engine (GpSimd per-op overhead gates the PE; +60us if moved).
 - Phase A is LDWEIGHTS-paced (108ns per FD-256 matmul, PE issue period
   109ns): its ~213us/block is the structural floor for FD=256; larger
   FD would need TB>512, which does not fit SBUF (hT + xc scale with TB).
 - Probes that REGRESSED from this configuration (all reverted): wc
   combos on GpSimd (+290us), C21/C12 combines on GpSimd (+62us), a
   one-step-lookahead prep pipeline for phase-B chunks (+330us), and
   host-shipping the Bc0 w1-combo as its own per-j stream (+115us). The
   emission order below is load-bearing via the Tile scheduler; measure
   before and after any change.

Compute dtype: bf16 operands with fp32 PSUM accumulation; Cacc bf16.
"""

import numpy as np
import ml_dtypes

import concourse.bass as bass
import concourse.mybir as mybir
import concourse.tile as tile
from concourse import bacc
from concourse.bass_utils import run_bass_kernel_spmd

P = 128
N_CORES = 8

TOKENS = 16384
HIDDEN = 2048
FFN = 8192

BF16 = mybir.dt.bfloat16
F32 = mybir.dt.float32

NP_BF16 = np.dtype(ml_dtypes.bfloat16)

# fp8e4 DoubleRow on part of GEMM2 was tried and REVERTED: numerics were
# exactly as predicted (rel err 1.36e-2 with 6/64 k-tiles) but mixing DR
# matmuls into the stream dropped the whole PE clock ~21% (216->260ns per
# FD-512 bf16 matmul), a global net loss of ~250us.

# Strassen products (0-indexed i = P1..P7):
#   Ac: 0:A11+A22 1:A21+A22 2:A11 3:A22 4:A11+A12 5:A21-A11 6:A12-A22
#   Bc: 0:B11+B22 1:B11    2:B12-B22 3:B21-B11 4:B22 5:B11+B12 6:B21+B22
#   C11 = P0+P3-P4+P6 ; C12 = P2+P4 ; C21 = P1+P3 ; C22 = P0-P1+P2+P5
# PE issue order: alias-weight products first (no combo dependency).
PORD = [4, 1, 0, 3, 2, 6, 5]

# GEMM2 product issue order: chosen so every product can drain into Cacc
# immediately at completion (first writer of each quadrant arrives first)
# and early quadrants (C12, C21) DMA out mid-phase.
PORD2 = [2, 4, 1, 3, 0, 6, 5]


def build_mlp(T, H, F, TB=512, n_cores=N_CORES,
              slab_bufs=3, wc_bufs=3, w2c_bufs=3, ac_bufs=3):
    """Per-core fused Strassen-GEMM1 + Strassen-GEMM2 graph (SPMD)."""
    KH = H // P            # 16 contraction tiles for GEMM1
    KHh = KH // 2          # 8 per k-half
    FT = F // P            # 64 ffn tiles
    FTh = FT // 2          # 32 Strassen column-pair iterations
    HT = H // P            # 16 hidden tiles
    KF = F // P            # 64 contraction tiles for GEMM2
    TH = TB // 2           # 256 token half
    KFh = KF // 2          # 32 ktiles per F-half (GEMM2 product contraction)
    CK = 4                 # GEMM2 w2c k-chunk (ktiles per streamed chunk)
    NCH = KFh // CK        # 8 chunks per product
    Hh = H // 2            # 1024 (quadrant H width)
    MT = TB // P           # 4 token m-tiles per block
    n_blocks = T // TB
    assert T % TB == 0 and H % (2 * P) == 0 and F % (2 * P) == 0

    nc = bacc.Bacc("TRN2", target_bir_lowering=False, debug=False,
                   num_devices=n_cores)

    BCOLS = -(-(FT + HT) // P) * P
    # All streamed tensors are packed on the host so each DMA chunk is
    # fully contiguous in DRAM (2KB-line strided layouts measured ~240GB/s
    # vs ~358 nominal; contiguous chunks stream at full rate).
    xc_d = nc.dram_tensor("xc", (n_blocks, P, 7 * KHh, TH), BF16,
                          kind="ExternalInput").ap()
    w1p_d = nc.dram_tensor("w1p", (FTh, P, KH, 2 * P), BF16,
                           kind="ExternalInput").ap()
    w2c_d = nc.dram_tensor("w2c", (7, NCH, P, CK, Hh), BF16,
                           kind="ExternalInput").ap()
    bc_d = nc.dram_tensor("bc", (P, BCOLS), F32, kind="ExternalInput").ap()
    cst_d = nc.dram_tensor("cst", (1, H + P), BF16, kind="ExternalInput").ap()
    out_d = nc.dram_tensor("out", (T, H), BF16, kind="ExternalOutput").ap()

    xc_r = xc_d.rearrange("t p c k -> p t c k")        # [128, nb, 7*KHh, TH]
    w1p_r = w1p_d.rearrange("j p k c -> p j k c")      # [128, FTh, KH, 2P]
    w2c_r = w2c_d.rearrange("i c p k h -> p i c k h")  # [128, 7, NCH, CK, Hh]
    out_r = out_d.rearrange("(t m p) h -> p t m h", p=P, m=MT)

    GELU = mybir.ActivationFunctionType.Gelu

    # GEMM2 A-side (h^T) piece sources in hT coords: (koff, toff) with
    # A11=(0,0) A12=(KFh,0) A21=(0,TH) A22=(KFh,TH).
    A11, A12, A21, A22 = (0, 0), (KFh, 0), (0, TH), (KFh, TH)
    AC_SRC = {
        0: (A11, A22, "add"),
        1: (A21, A22, "add"),
        2: (A11, None, None),
        3: (A22, None, None),
        4: (A11, A12, "add"),
        5: (A21, A11, "sub"),
        6: (A12, A22, "sub"),
    }
    # Quadrant -> (m-tile base, H col base) in Cacc / out coords.
    QUAD = {"C11": (0, 0), "C12": (0, Hh), "C21": (2, 0), "C22": (2, Hh)}
    # Per product (in whatever order issued): list of (quad, mode) where
    # mode: "cp" = first-writer copy, "ncp" = first-writer negate-copy,
    # "add"/"sub" = in-place accumulate. Quadrants complete in the order
    # C12 (after P4), C21 (after P3), C11 (after P6), C22 (after P5).
    DRAIN = {
        2: [("C12", "cp"), ("C22", "cp")],
        4: [("C12", "add"), ("C11", "ncp")],
        1: [("C21", "cp"), ("C22", "sub")],
        3: [("C21", "add"), ("C11", "add")],
        0: [("C11", "add"), ("C22", "add")],
        6: [("C11", "add")],
        5: [("C22", "add")],
    }
    DONE_AFTER = {4: "C12", 3: "C21", 6: "C11", 5: "C22"}
    BIAS_OFF = {2: Hh, 3: 0}   # P2 feeds H-half-2 quadrants, P3 H-half-1

    with tile.TileContext(nc) as tc:
        with (
            tc.tile_pool(name="const", bufs=1) as const_pool,
            tc.tile_pool(name="xc", bufs=1) as xc_pool,
            tc.tile_pool(name="slab", bufs=slab_bufs) as slab_pool,
            tc.tile_pool(name="wc", bufs=wc_bufs) as wc_pool,
            tc.tile_pool(name="cp", bufs=2) as cp_pool,
            tc.tile_pool(name="cs", bufs=1) as cs_pool,
            tc.tile_pool(name="ht", bufs=1) as ht_pool,
            tc.tile_pool(name="ac", bufs=ac_bufs) as ac_pool,
            tc.tile_pool(name="w2c", bufs=w2c_bufs) as w2c_pool,
            tc.tile_pool(name="cacc", bufs=1) as cacc_pool,
            tc.tile_pool(name="ps", bufs=8, space="PSUM") as ps_pool,
        ):
            bc = const_pool.tile([P, BCOLS], F32)
            b1t = bc[:, 0:FT]
            cst = const_pool.tile([1, H + P], BF16)

            hT = ht_pool.tile([P, FT, TB], BF16)

            def emit_combos(wc, slab):
                B11 = slab[:, 0:KHh, 0:P]
                B12 = slab[:, 0:KHh, P:2 * P]
                B21 = slab[:, KHh:KH, 0:P]
                B22 = slab[:, KHh:KH, P:2 * P]
                nc.vector.tensor_add(wc[:, 0], B11, B22)   # Bc0 (P1)
                nc.vector.tensor_sub(wc[:, 1], B12, B22)   # Bc2 (P3)
                nc.vector.tensor_sub(wc[:, 2], B21, B11)   # Bc3 (P4)
                nc.vector.tensor_add(wc[:, 3], B11, B12)   # Bc5 (P6)
                nc.vector.tensor_add(wc[:, 4], B21, B22)   # Bc6 (P7)

            def slab_dma(j):
                s = slab_pool.tile([P, KH, 2 * P], BF16, tag="slab",
                                   name="slab")
                nc.sync.dma_start(out=s[:], in_=w1p_r[:, j])
                return s

            def xc_dma(xct, tt, i):
                nc.sync.dma_start(out=xct[:, i * KHh:(i + 1) * KHh, :],
                                  in_=xc_r[:, tt, i * KHh:(i + 1) * KHh, :])

            xc = None
            slabs = None
            for t in range(n_blocks):
                # block prologue: slabs for j=0,1 + combos for j=0 + xc.
                # For t>0 the prologue DMAs were already issued, spread
                # between the previous block's phase-B products (so they
                # land before phase A needs them instead of queueing
                # behind the whole w2c stream).
                if t == 0:
                    # cold start: the first matmul (P5, k=0) needs only
                    # slab row KHh and xc row 4*KHh — land those first,
                    # then stream the rest in consumption order.
                    xc = xc_pool.tile([P, 7 * KHh, TH], BF16, tag="xc",
                                      name="xc")
                    slab0 = slab_pool.tile([P, KH, 2 * P], BF16, tag="slab",
                                           name="slab")
                    nc.sync.dma_start(out=slab0[:, KHh:KHh + 1, :],
                                      in_=w1p_r[:, 0, KHh:KHh + 1, :])
                    nc.sync.dma_start(out=xc[:, 4 * KHh:4 * KHh + 1, :],
                                      in_=xc_r[:, 0, 4 * KHh:4 * KHh + 1, :])
                    nc.sync.dma_start(out=slab0[:, KHh + 1:KH, :],
                                      in_=w1p_r[:, 0, KHh + 1:KH, :])
                    nc.sync.dma_start(out=xc[:, 4 * KHh + 1:5 * KHh, :],
                                      in_=xc_r[:, 0, 4 * KHh + 1:5 * KHh, :])
                    nc.sync.dma_start(out=slab0[:, 0:KHh, :],
                                      in_=w1p_r[:, 0, 0:KHh, :])
                    xc_dma(xc, 0, 1)
                    xc_dma(xc, 0, 0)
                    slabs = [slab0, slab_dma(1)]
                    nc.sync.dma_start(out=bc[:], in_=bc_d[:])
                    nc.sync.dma_start(out=cst[:], in_=cst_d[:])
                    for i in [3, 2, 6, 5]:
                        xc_dma(xc, 0, i)
                wc0 = wc_pool.tile([P, 5, KHh, P], BF16, tag="wc", name="wc")
                emit_combos(wc0, slabs[0])
                wc1 = wc_pool.tile([P, 5, KHh, P], BF16, tag="wc", name="wc")
                emit_combos(wc1, slabs[1])
                wcs = [wc0, wc1]

                # ---- phase A: Strassen GEMM1, gelu -> hT ----
                for j in range(FTh):
                    # software pipeline: fetch slab j+2 and combo it
                    # immediately (2-j lookahead) so the in-order DVE —
                    # serialized behind the cp->cs->gelu chain — never
                    # blocks the PE at a j boundary.
                    if j + 2 < FTh:
                        slabs.append(slab_dma(j + 2))
                        wc_n = wc_pool.tile([P, 5, KHh, P], BF16, tag="wc",
                                            name="wc")
                        emit_combos(wc_n, slabs[2])
                        wcs.append(wc_n)
                    slab, wc = slabs[0], wcs[0]
                    B11 = slab[:, 0:KHh, 0:P]
                    B22 = slab[:, KHh:KH, P:2 * P]
                    lhs = {0: wc[:, 0], 1: B11, 2: wc[:, 1], 3: wc[:, 2],
                           4: B22, 5: wc[:, 3], 6: wc[:, 4]}

                    psA = ps_pool.tile([P, TB], F32, tag="ps")  # P5 | P2
                    psB = ps_pool.tile([P, TB], F32, tag="ps")  # P1 | P4
                    psC = ps_pool.tile([P, TB], F32, tag="ps")  # P3 | P7
                    psD = ps_pool.tile([P, TB], F32, tag="ps")  # P6 | --
                    pslot = {4: (psA, 0), 1: (psA, 1), 0: (psB, 0),
                             3: (psB, 1), 2: (psC, 0), 6: (psC, 1),
                             5: (psD, 0)}
                    cp = cp_pool.tile([P, 7, TH], BF16, tag="cp")
                    for i in PORD:
                        pst, half = pslot[i]
                        dst = pst[:, half * TH:(half + 1) * TH]
                        for k in range(KHh):
                            nc.tensor.matmul(
                                dst, lhsT=lhs[i][:, k, :],
                                rhs=xc[:, i * KHh + k, :],
                                start=(k == 0), stop=(k == KHh - 1))
                        nc.scalar.copy(cp[:, i, :], dst)

                    cs = cs_pool.tile([P, 8, TH], BF16, tag="cs")
                    # DVE: C11 chain + C21 + C12
                    nc.vector.tensor_add(cs[:, 0], cp[:, 0], cp[:, 3])
                    nc.vector.tensor_sub(cs[:, 1], cs[:, 0], cp[:, 4])
                    nc.vector.tensor_add(cs[:, 2], cs[:, 1], cp[:, 6])  # C11
                    nc.vector.tensor_add(cs[:, 3], cp[:, 1], cp[:, 3])  # C21
                    nc.vector.tensor_add(cs[:, 4], cp[:, 2], cp[:, 4])  # C12
                    # Pool: C22 chain
                    nc.gpsimd.tensor_sub(cs[:, 5], cp[:, 0], cp[:, 1])
                    nc.gpsimd.tensor_add(cs[:, 6], cs[:, 5], cp[:, 2])
                    nc.gpsimd.tensor_add(cs[:, 7], cs[:, 6], cp[:, 5])  # C22

                    nc.scalar.activation(hT[:, j, 0:TH], cs[:, 2], GELU,
                                         bias=b1t[:, j:j + 1])
                    nc.scalar.activation(hT[:, j, TH:TB], cs[:, 3], GELU,
                                         bias=b1t[:, j:j + 1])
                    nc.scalar.activation(hT[:, FTh + j, 0:TH], cs[:, 4], GELU,
                                         bias=b1t[:, FTh + j:FTh + j + 1])
                    nc.scalar.activation(hT[:, FTh + j, TH:TB], cs[:, 7], GELU,
                                         bias=b1t[:, FTh + j:FTh + j + 1])
                    slabs.pop(0)
                    wcs.pop(0)

                # ---- phase B: Strassen GEMM2 (transposed), y -> out ----
                cacc = cacc_pool.tile([P, MT, H], BF16, tag="cacc")
                ones = cst[0:1, H:H + P]
                nxt = t + 1 < n_blocks
                for idx, i in enumerate(PORD2):
                    # spread next block's prologue DMAs between products so
                    # they interleave with (not trail) the w2c stream.
                    if nxt:
                        if idx == 0:
                            xc_n = xc_pool.tile([P, 7 * KHh, TH], BF16,
                                                tag="xc", name="xc")
                            xc_dma(xc_n, t + 1, 4)
                        elif idx == 1:
                            xc_dma(xc_n, t + 1, 1)
                            xc_dma(xc_n, t + 1, 0)
                        elif idx == 2:
                            slabs_n = [slab_dma(0)]
                        elif idx == 3:
                            xc_dma(xc_n, t + 1, 3)
                            xc_dma(xc_n, t + 1, 2)
                        elif idx == 4:
                            slabs_n.append(slab_dma(1))
                        elif idx == 5:
                            xc_dma(xc_n, t + 1, 6)
                            xc_dma(xc_n, t + 1, 5)
                    # 4 PSUM banks per product: index 2*m + fd.
                    ps = [ps_pool.tile([P, 512], F32, tag="ps", name="ps2")
                          for _ in range(4)]
                    has_bias = i in BIAS_OFF
                    if has_bias:
                        boff = BIAS_OFF[i]
                        for m in range(2):
                            for fd in range(2):
                                nc.tensor.matmul(
                                    ps[2 * m + fd][:], lhsT=ones,
                                    rhs=cst[0:1, boff + fd * 512:
                                            boff + (fd + 1) * 512],
                                    start=True, stop=False)
                    (ak, at), src2, op = AC_SRC[i]
                    for c in range(NCH):
                        w2cc = w2c_pool.tile([P, CK, Hh], BF16, tag="w2c")
                        nc.sync.dma_start(out=w2cc[:], in_=w2c_r[:, i, c])
                        ks = slice(ak + c * CK, ak + (c + 1) * CK)
                        if op is None:
                            lhsp = hT[:, ks, at:at + TH]
                        else:
                            bk, bt = src2
                            ks2 = slice(bk + c * CK, bk + (c + 1) * CK)
                            lhsp = ac_pool.tile([P, CK, TH], BF16, tag="ac")
                            if op == "add":
                                nc.vector.tensor_add(
                                    lhsp[:], hT[:, ks, at:at + TH],
                                    hT[:, ks2, bt:bt + TH])
                            else:
                                nc.vector.tensor_sub(
                                    lhsp[:], hT[:, ks, at:at + TH],
                                    hT[:, ks2, bt:bt + TH])
                        for kk in range(CK):
                            for m in range(2):
                                for fd in range(2):
                                    nc.tensor.matmul(
                                        ps[2 * m + fd][:],
                                        lhsT=lhsp[:, kk, m * P:(m + 1) * P],
                                        rhs=w2cc[:, kk, fd * 512:
                                                 (fd + 1) * 512],
                                        start=(c == 0 and kk == 0
                                               and not has_bias),
                                        stop=(c == NCH - 1 and kk == CK - 1))
                    # drain into Cacc, fused +/-.
                    for quad, mode in DRAIN[i]:
                        qm0, qh0 = QUAD[quad]
                        for m in range(2):
                            for fd in range(2):
                                dst = cacc[:, qm0 + m,
                                           qh0 + fd * 512:qh0 + (fd + 1) * 512]
                                src = ps[2 * m + fd][:]
                                if mode == "cp":
                                    nc.scalar.copy(dst, src)
                                elif mode == "ncp":
                                    nc.scalar.mul(dst, src, -1.0)
                                elif mode == "add":
                                    nc.vector.tensor_add(dst, dst, src)
                                else:
                                    nc.vector.tensor_sub(dst, dst, src)
                    if i in DONE_AFTER:
                        qm0, qh0 = QUAD[DONE_AFTER[i]]
                        for m in range(2):
                            nc.sync.dma_start(
                                out=out_r[:, t, qm0 + m, qh0:qh0 + Hh],
                                in_=cacc[:, qm0 + m, qh0:qh0 + Hh])
                if nxt:
                    xc, slabs = xc_n, slabs_n

    nc.compile()
    return nc


def make_in_maps(x, w1, b1, w2, b2, n_cores=N_CORES):
    """Shard FULL f32 inputs into per-core in_maps (host-side layout prep)."""
    T_core = x.shape[0] // n_cores
    H = x.shape[1]
    F = w1.shape[1]
    FT = F // P
    HT = w2.shape[1] // P
    KH_ = H // P
    TB = 512
    TH = TB // 2
    n_blocks = T_core // TB
    Hh = H // 2
    Fh = F // 2

    # w1 permuted so that Strassen column-pair (j of F-half1, j of F-half2)
    # is a contiguous 256-col slab: w1p[:, j*256 + s*128 + c] = w1[:, s*F/2 + j*128 + c]
    # Then repacked per-slab-contiguous: w1ps[j, p, k, c] = w1p[k*128+p, j*256+c].
    w1p = (
        w1.reshape(H, 2, F // 256, 128).transpose(0, 2, 1, 3).reshape(H, F)
    ).astype(NP_BF16)
    w1ps = np.ascontiguousarray(
        w1p.reshape(KH_, P, F // 256, 256).transpose(2, 1, 0, 3))

    # GEMM2 B-side Strassen pieces of w2 (combined in f32, then bf16),
    # repacked per-chunk-contiguous: [i, kc, p, kk, h].
    B11 = w2[0:Fh, 0:Hh]
    B12 = w2[0:Fh, Hh:H]
    B21 = w2[Fh:F, 0:Hh]
    B22 = w2[Fh:F, Hh:H]
    w2c = np.stack([B11 + B22, B11, B12 - B22, B21 - B11, B22,
                    B11 + B12, B21 + B22]).astype(NP_BF16)
    w2c = np.ascontiguousarray(
        w2c.reshape(7, Fh // P // 4, 4, P, Hh).transpose(0, 1, 3, 2, 4))

    b1t = b1.astype(np.float32).reshape(FT, P).T
    b2t = b2.astype(np.float32).reshape(HT, P).T
    BCOLS = -(-(FT + HT) // P) * P
    bcm = np.zeros((P, BCOLS), dtype=np.float32)
    bcm[:, 0:FT] = b1t
    bcm[:, FT:FT + HT] = b2t

    cst = np.zeros((1, H + P), dtype=NP_BF16)
    cst[0, 0:H] = b2.astype(NP_BF16)
    cst[0, H:H + P] = 1.0

    in_maps = []
    for c in range(n_cores):
        xs = x[c * T_core:(c + 1) * T_core].astype(np.float32)
        # block-contiguous layout: xc[b, p, i*8+kk, tok] = piece_i[kk*128+p, tok]
        xc = np.empty((n_blocks, P, 7 * Hh // P, TH), dtype=NP_BF16)
        for b in range(n_blocks):
            xb = xs[b * TB:(b + 1) * TB]
            A11 = xb[0:TH, 0:Hh]
            A12 = xb[0:TH, Hh:H]
            A21 = xb[TH:TB, 0:Hh]
            A22 = xb[TH:TB, Hh:H]
            combos = (A11 + A22, A21 + A22, A11, A22,
                      A11 + A12, A21 - A11, A12 - A22)
            for i, S in enumerate(combos):
                piece = S.T.astype(NP_BF16).reshape(Hh // P, P, TH)
                xc[b, :, i * (Hh // P):(i + 1) * (Hh // P), :] = \
                    piece.transpose(1, 0, 2)
        in_maps.append({"xc": xc, "w1p": w1ps, "w2c": w2c, "bc": bcm,
                        "cst": cst})
    return in_maps


_CACHE = {}


def _get_nc():
    if "nc" not in _CACHE:
        _CACHE["nc"] = build_mlp(TOKENS // N_CORES, HIDDEN, FFN, TB=512)
    return _CACHE["nc"]


def run(x, w1, b1, w2, b2, trace=False, **kw):
    nc = _get_nc()
    in_maps = make_in_maps(x, w1, b1, w2, b2)
    res = run_bass_kernel_spmd(nc, in_maps, core_ids=list(range(N_CORES)),
                               trace=trace, **kw)
    y = np.concatenate(
        [np.asarray(res.results[i]["out"]) for i in range(N_CORES)], axis=0)
    return np.ascontiguousarray(y.astype(np.float32)), res


def kernel(x, w1, b1, w2, b2):
    x = np.asarray(x, dtype=np.float32)
    w1 = np.asarray(w1, dtype=np.float32)
    b1 = np.asarray(b1, dtype=np.float32)
    w2 = np.asarray(w2, dtype=np.float32)
    b2 = np.asarray(b2, dtype=np.float32)
    y, _ = run(x, w1, b1, w2, b2, trace=False)
    return y


# revision 28
# speedup vs baseline: 1.0618x; 1.0618x over previous
"""Trainium2 Bass kernel for nn_AllToAllMoE (degenerate single-group MoE == dense MLP).

reference:  y = gelu(x @ w1 + b1, exact) @ w2 + b2
  x  (16384, 2048) f32
  w1 (2048, 8192) f32, b1 (8192,) f32
  w2 (8192, 2048) f32, b2 (2048,) f32

Strategy: the all_to_all with a single replica group is an identity permutation,
so the problem is a dense 2-layer MLP. TOKENS are sharded across the 8
NeuronCores (data parallel, no collectives). GEMM1 is computed in the
transposed orientation (ffn dim on partitions, tokens on the free dim);
GEMM2 is computed in the natural orientation (tokens on partitions, hidden
on the free dim) so its products run at FD=512 and the output DMAs out
row-major [T, H] with no host transpose.

GEMM1 (x @ w1) uses one level of Strassen per 512-token block:
  tokens 512 -> halves 256, K 2048 -> halves 1024, F 8192 -> halves 4096.
  7 products P_i = Ac_i @ Bc_i accumulate in PSUM (FD=256, two products
  packed per 2KB PSUM bank). A-side combos (of x) are precomputed on the
  host and shipped as `xc`; B-side combos (of w1) are built on-chip by the
  Vector engine from a column-pair slab of the permuted `w1p`; products are
  drained PSUM->SBUF (bf16) by the Scalar engine; the C-quadrant combines
  run on Vector+GpSimd(Pool) in bf16; ScalarE applies exact GELU + b1.

GEMM2 (h @ w2) uses one level of Strassen as well, in transposed-output
form:  y[T, H] = h^T[T, F] @ w2[F, H], lhsT = hT pieces (A-side combos of
h quadrants built on the fly by the Vector engine from hT), rhs = w2
quadrant combos precomputed on the HOST and shipped as `w2c` (7/4 x w2
bytes, streamed per block). Products accumulate in 4 PSUM banks (FD=512,
so the PE - not LDWEIGHTS or DMA - is the pacer), are drained with fused
+/- adds into a bf16 accumulator Cacc[T-part, H], and each output
quadrant DMAs out (bf16, host converts to f32) as soon as its last
product lands. b2 is injected via K=1 matmuls (ones (x) b2row) into the
PSUM init of products P2 and P3 (which together touch each output
quadrant exactly once, always with + sign).

Perf-critical details learned on HW (do not regress):
 - Every streamed tensor (w1p slabs, w2c chunks, xc blocks) is packed
   host-side so each DMA transfer is fully contiguous in DRAM; strided
   2KB-line layouts cap at ~240GB/s vs ~300+ contiguous, which starves
   phase B (w2c needs ~294GB/s sustained to keep the PE fed).
 - Next block's xc/slab prologue DMAs are spread between phase-B
   products; issued all-at-once after phase B they queue behind 58MB of
   w2c and arrive after phase A has started (~12us/block of PE stalls).
 - slab_bufs=3: with 2, the slab(j+2) DMA start serializes behind slab(j)
   reads and w1 combos go late (~60us/block). wc combos must stay on the
   Vector engine (GpSimd per-op overhead gates the PE; +60us if moved).
 - Phase A is LDWEIGHTS-paced (108ns per FD-256 matmul, PE issue period
   109ns): its ~213us/block is the structural floor for FD=256; larger
   FD would need TB>512, which does not fit SBUF (hT + xc scale with TB).
 - Probes that REGRESSED from this configuration (all reverted): wc
   combos on GpSimd (+290us), C21/C12 combines on GpSimd (+62us), a
   one-step-lookahead prep pipeline for phase-B chunks (+330us), and
   host-shipping the Bc0 w1-combo as its own per-j stream (+115us), and
   2-j wc-combo lookahead (wc_bufs=3) funded by w2c_bufs 4->3 (+93us). The
   emission order below is load-bearing via the Tile scheduler; measure
   before and after any change.

Compute dtype: bf16 operands with fp32 PSUM accumulation; Cacc bf16.
"""

import numpy as np
import ml_dtypes

import concourse.bass as bass
import concourse.mybir as mybir
import concourse.tile as tile
from concourse import bacc
from concourse.bass_utils import run_bass_kernel_spmd

P = 128
N_CORES = 8

TOKENS = 16384
HIDDEN = 2048
FFN = 8192

BF16 = mybir.dt.bfloat16
F32 = mybir.dt.float32

NP_BF16 = np.dtype(ml_dtypes.bfloat16)

# fp8e4 DoubleRow on part of GEMM2 was tried and REVERTED: numerics were
# exactly as predicted (rel err 1.36e-2 with 6/64 k-tiles) but mixing DR
# matmuls into the stream dropped the whole PE clock ~21% (216->260ns per
# FD-512 bf16 matmul), a global net loss of ~250us.

# Strassen products (0-indexed i = P1..P7):
#   Ac: 0:A11+A22 1:A21+A22 2:A11 3:A22 4:A11+A12 5:A21-A11 6:A12-A22
#   Bc: 0:B11+B22 1:B11    2:B12-B22 3:B21-B11 4:B22 5:B11+B12 6:B21+B22
#   C11 = P0+P3-P4+P6 ; C12 = P2+P4 ; C21 = P1+P3 ; C22 = P0-P1+P2+P5
# PE issue order: alias-weight products first (no combo dependency).
PORD = [4, 1, 0, 3, 2, 6, 5]

# GEMM2 product issue order: chosen so every product can drain into Cacc
# immediately at completion (first writer of each quadrant arrives first)
# and early quadrants (C12, C21) DMA out mid-phase.
PORD2 = [2, 4, 1, 3, 0, 6, 5]


def build_mlp(T, H, F, TB=512, n_cores=N_CORES,
              slab_bufs=3, wc_bufs=2, w2c_bufs=4, ac_bufs=3):
    """Per-core fused Strassen-GEMM1 + Strassen-GEMM2 graph (SPMD)."""
    KH = H // P            # 16 contraction tiles for GEMM1
    KHh = KH // 2          # 8 per k-half
    FT = F // P            # 64 ffn tiles
    FTh = FT // 2          # 32 Strassen column-pair iterations
    HT = H // P            # 16 hidden tiles
    KF = F // P            # 64 contraction tiles for GEMM2
    TH = TB // 2           # 256 token half
    KFh = KF // 2          # 32 ktiles per F-half (GEMM2 product contraction)
    CK = 4                 # GEMM2 w2c k-chunk (ktiles per streamed chunk)
    NCH = KFh // CK        # 8 chunks per product
    Hh = H // 2            # 1024 (quadrant H width)
    MT = TB // P           # 4 token m-tiles per block
    n_blocks = T // TB
    assert T % TB == 0 and H % (2 * P) == 0 and F % (2 * P) == 0

    nc = bacc.Bacc("TRN2", target_bir_lowering=False, debug=False,
                   num_devices=n_cores)

    BCOLS = -(-(FT + HT) // P) * P
    # All streamed tensors are packed on the host so each DMA chunk is
    # fully contiguous in DRAM (2KB-line strided layouts measured ~240GB/s
    # vs ~358 nominal; contiguous chunks stream at full rate).
    xc_d = nc.dram_tensor("xc", (n_blocks, P, 7 * KHh, TH), BF16,
                          kind="ExternalInput").ap()
    w1p_d = nc.dram_tensor("w1p", (FTh, P, KH, 2 * P), BF16,
                           kind="ExternalInput").ap()
    w2c_d = nc.dram_tensor("w2c", (7, NCH, P, CK, Hh), BF16,
                           kind="ExternalInput").ap()
    bc_d = nc.dram_tensor("bc", (P, BCOLS), F32, kind="ExternalInput").ap()
    cst_d = nc.dram_tensor("cst", (1, H + P), BF16, kind="ExternalInput").ap()
    out_d = nc.dram_tensor("out", (T, H), BF16, kind="ExternalOutput").ap()

    xc_r = xc_d.rearrange("t p c k -> p t c k")        # [128, nb, 7*KHh, TH]
    w1p_r = w1p_d.rearrange("j p k c -> p j k c")      # [128, FTh, KH, 2P]
    w2c_r = w2c_d.rearrange("i c p k h -> p i c k h")  # [128, 7, NCH, CK, Hh]
    out_r = out_d.rearrange("(t m p) h -> p t m h", p=P, m=MT)

    GELU = mybir.ActivationFunctionType.Gelu

    # GEMM2 A-side (h^T) piece sources in hT coords: (koff, toff) with
    # A11=(0,0) A12=(KFh,0) A21=(0,TH) A22=(KFh,TH).
    A11, A12, A21, A22 = (0, 0), (KFh, 0), (0, TH), (KFh, TH)
    AC_SRC = {
        0: (A11, A22, "add"),
        1: (A21, A22, "add"),
        2: (A11, None, None),
        3: (A22, None, None),
        4: (A11, A12, "add"),
        5: (A21, A11, "sub"),
        6: (A12, A22, "sub"),
    }
    # Quadrant -> (m-tile base, H col base) in Cacc / out coords.
    QUAD = {"C11": (0, 0), "C12": (0, Hh), "C21": (2, 0), "C22": (2, Hh)}
    # Per product (in whatever order issued): list of (quad, mode) where
    # mode: "cp" = first-writer copy, "ncp" = first-writer negate-copy,
    # "add"/"sub" = in-place accumulate. Quadrants complete in the order
    # C12 (after P4), C21 (after P3), C11 (after P6), C22 (after P5).
    DRAIN = {
        2: [("C12", "cp"), ("C22", "cp")],
        4: [("C12", "add"), ("C11", "ncp")],
        1: [("C21", "cp"), ("C22", "sub")],
        3: [("C21", "add"), ("C11", "add")],
        0: [("C11", "add"), ("C22", "add")],
        6: [("C11", "add")],
        5: [("C22", "add")],
    }
    DONE_AFTER = {4: "C12", 3: "C21", 6: "C11", 5: "C22"}
    BIAS_OFF = {2: Hh, 3: 0}   # P2 feeds H-half-2 quadrants, P3 H-half-1

    with tile.TileContext(nc) as tc:
        with (
            tc.tile_pool(name="const", bufs=1) as const_pool,
            tc.tile_pool(name="xc", bufs=1) as xc_pool,
            tc.tile_pool(name="slab", bufs=slab_bufs) as slab_pool,
            tc.tile_pool(name="wc", bufs=wc_bufs) as wc_pool,
            tc.tile_pool(name="cp", bufs=2) as cp_pool,
            tc.tile_pool(name="cs", bufs=1) as cs_pool,
            tc.tile_pool(name="ht", bufs=1) as ht_pool,
            tc.tile_pool(name="ac", bufs=ac_bufs) as ac_pool,
            tc.tile_pool(name="w2c", bufs=w2c_bufs) as w2c_pool,
            tc.tile_pool(name="cacc", bufs=1) as cacc_pool,
            tc.tile_pool(name="ps", bufs=8, space="PSUM") as ps_pool,
        ):
            bc = const_pool.tile([P, BCOLS], F32)
            b1t = bc[:, 0:FT]
            cst = const_pool.tile([1, H + P], BF16)

            hT = ht_pool.tile([P, FT, TB], BF16)

            def emit_combos(wc, slab):
                B11 = slab[:, 0:KHh, 0:P]
                B12 = slab[:, 0:KHh, P:2 * P]
                B21 = slab[:, KHh:KH, 0:P]
                B22 = slab[:, KHh:KH, P:2 * P]
                nc.vector.tensor_add(wc[:, 0], B11, B22)   # Bc0 (P1)
                nc.vector.tensor_sub(wc[:, 1], B12, B22)   # Bc2 (P3)
                nc.vector.tensor_sub(wc[:, 2], B21, B11)   # Bc3 (P4)
                nc.vector.tensor_add(wc[:, 3], B11, B12)   # Bc5 (P6)
                nc.vector.tensor_add(wc[:, 4], B21, B22)   # Bc6 (P7)

            def slab_dma(j):
                s = slab_pool.tile([P, KH, 2 * P], BF16, tag="slab",
                                   name="slab")
                nc.sync.dma_start(out=s[:], in_=w1p_r[:, j])
                return s

            def xc_dma(xct, tt, i):
                nc.sync.dma_start(out=xct[:, i * KHh:(i + 1) * KHh, :],
                                  in_=xc_r[:, tt, i * KHh:(i + 1) * KHh, :])

            xc = None
            slabs = None
            for t in range(n_blocks):
                # block prologue: slabs for j=0,1 + combos for j=0 + xc.
                # For t>0 the prologue DMAs were already issued, spread
                # between the previous block's phase-B products (so they
                # land before phase A needs them instead of queueing
                # behind the whole w2c stream).
                if t == 0:
                    # cold start: the first matmul (P5, k=0) needs only
                    # slab row KHh and xc row 4*KHh — land those first,
                    # then stream the rest in consumption order.
                    xc = xc_pool.tile([P, 7 * KHh, TH], BF16, tag="xc",
                                      name="xc")
                    slab0 = slab_pool.tile([P, KH, 2 * P], BF16, tag="slab",
                                           name="slab")
                    nc.sync.dma_start(out=slab0[:, KHh:KHh + 1, :],
                                      in_=w1p_r[:, 0, KHh:KHh + 1, :])
                    nc.sync.dma_start(out=xc[:, 4 * KHh:4 * KHh + 1, :],
                                      in_=xc_r[:, 0, 4 * KHh:4 * KHh + 1, :])
                    nc.sync.dma_start(out=slab0[:, KHh + 1:KH, :],
                                      in_=w1p_r[:, 0, KHh + 1:KH, :])
                    nc.sync.dma_start(out=xc[:, 4 * KHh + 1:5 * KHh, :],
                                      in_=xc_r[:, 0, 4 * KHh + 1:5 * KHh, :])
                    nc.sync.dma_start(out=slab0[:, 0:KHh, :],
                                      in_=w1p_r[:, 0, 0:KHh, :])
                    xc_dma(xc, 0, 1)
                    xc_dma(xc, 0, 0)
                    slabs = [slab0, slab_dma(1)]
                    nc.sync.dma_start(out=bc[:], in_=bc_d[:])
                    nc.sync.dma_start(out=cst[:], in_=cst_d[:])
                    for i in [3, 2, 6, 5]:
                        xc_dma(xc, 0, i)
                wc0 = wc_pool.tile([P, 5, KHh, P], BF16, tag="wc", name="wc")
                emit_combos(wc0, slabs[0])
                wcs = [wc0]

                # ---- phase A: Strassen GEMM1, gelu -> hT ----
                for j in range(FTh):
                    # software pipeline: fetch slab j+2, combo slab j+1 so
                    # the in-order DVE never blocks the PE at a j boundary.
                    if j + 2 < FTh:
                        slabs.append(slab_dma(j + 2))
                    if j + 1 < FTh:
                        wc_n = wc_pool.tile([P, 5, KHh, P], BF16, tag="wc",
                                            name="wc")
                        emit_combos(wc_n, slabs[1])
                        wcs.append(wc_n)
                    slab, wc = slabs[0], wcs[0]
                    B11 = slab[:, 0:KHh, 0:P]
                    B22 = slab[:, KHh:KH, P:2 * P]
                    lhs = {0: wc[:, 0], 1: B11, 2: wc[:, 1], 3: wc[:, 2],
                           4: B22, 5: wc[:, 3], 6: wc[:, 4]}

                    psA = ps_pool.tile([P, TB], F32, tag="ps")  # P5 | P2
                    psB = ps_pool.tile([P, TB], F32, tag="ps")  # P1 | P4
                    psC = ps_pool.tile([P, TB], F32, tag="ps")  # P3 | P7
                    psD = ps_pool.tile([P, TB], F32, tag="ps")  # P6 | --
                    pslot = {4: (psA, 0), 1: (psA, 1), 0: (psB, 0),
                             3: (psB, 1), 2: (psC, 0), 6: (psC, 1),
                             5: (psD, 0)}
                    cp = cp_pool.tile([P, 7, TH], BF16, tag="cp")
                    for i in PORD:
                        pst, half = pslot[i]
                        dst = pst[:, half * TH:(half + 1) * TH]
                        for k in range(KHh):
                            nc.tensor.matmul(
                                dst, lhsT=lhs[i][:, k, :],
                                rhs=xc[:, i * KHh + k, :],
                                start=(k == 0), stop=(k == KHh - 1))
                        nc.scalar.copy(cp[:, i, :], dst)

                    cs = cs_pool.tile([P, 8, TH], BF16, tag="cs")
                    # DVE: C11 chain + C21 + C12
                    nc.vector.tensor_add(cs[:, 0], cp[:, 0], cp[:, 3])
                    nc.vector.tensor_sub(cs[:, 1], cs[:, 0], cp[:, 4])
                    nc.vector.tensor_add(cs[:, 2], cs[:, 1], cp[:, 6])  # C11
                    nc.vector.tensor_add(cs[:, 3], cp[:, 1], cp[:, 3])  # C21
                    nc.vector.tensor_add(cs[:, 4], cp[:, 2], cp[:, 4])  # C12
                    # Pool: C22 chain
                    nc.gpsimd.tensor_sub(cs[:, 5], cp[:, 0], cp[:, 1])
                    nc.gpsimd.tensor_add(cs[:, 6], cs[:, 5], cp[:, 2])
                    nc.gpsimd.tensor_add(cs[:, 7], cs[:, 6], cp[:, 5])  # C22

                    nc.scalar.activation(hT[:, j, 0:TH], cs[:, 2], GELU,
                                         bias=b1t[:, j:j + 1])
                    nc.scalar.activation(hT[:, j, TH:TB], cs[:, 3], GELU,
                                         bias=b1t[:, j:j + 1])
                    nc.scalar.activation(hT[:, FTh + j, 0:TH], cs[:, 4], GELU,
                                         bias=b1t[:, FTh + j:FTh + j + 1])
                    nc.scalar.activation(hT[:, FTh + j, TH:TB], cs[:, 7], GELU,
                                         bias=b1t[:, FTh + j:FTh + j + 1])
                    slabs.pop(0)
                    wcs.pop(0)

                # ---- phase B: Strassen GEMM2 (transposed), y -> out ----
                cacc = cacc_pool.tile([P, MT, H], BF16, tag="cacc")
                ones = cst[0:1, H:H + P]
                nxt = t + 1 < n_blocks
                for idx, i in enumerate(PORD2):
                    # spread next block's prologue DMAs between products so
                    # they interleave with (not trail) the w2c stream.
                    if nxt:
                        if idx == 0:
                            xc_n = xc_pool.tile([P, 7 * KHh, TH], BF16,
                                                tag="xc", name="xc")
                            xc_dma(xc_n, t + 1, 4)
                        elif idx == 1:
                            xc_dma(xc_n, t + 1, 1)
                            xc_dma(xc_n, t + 1, 0)
                        elif idx == 2:
                            slabs_n = [slab_dma(0)]
                        elif idx == 3:
                            xc_dma(xc_n, t + 1, 3)
                            xc_dma(xc_n, t + 1, 2)
                        elif idx == 4:
                            slabs_n.append(slab_dma(1))
                        elif idx == 5:
                            xc_dma(xc_n, t + 1, 6)
                            xc_dma(xc_n, t + 1, 5)
                    # 4 PSUM banks per product: index 2*m + fd.
                    ps = [ps_pool.tile([P, 512], F32, tag="ps", name="ps2")
                          for _ in range(4)]
                    has_bias = i in BIAS_OFF
                    if has_bias:
                        boff = BIAS_OFF[i]
                        for m in range(2):
                            for fd in range(2):
                                nc.tensor.matmul(
                                    ps[2 * m + fd][:], lhsT=ones,
                                    rhs=cst[0:1, boff + fd * 512:
                                            boff + (fd + 1) * 512],
                                    start=True, stop=False)
                    (ak, at), src2, op = AC_SRC[i]
                    for c in range(NCH):
                        w2cc = w2c_pool.tile([P, CK, Hh], BF16, tag="w2c")
                        nc.sync.dma_start(out=w2cc[:], in_=w2c_r[:, i, c])
                        ks = slice(ak + c * CK, ak + (c + 1) * CK)
                        if op is None:
                            lhsp = hT[:, ks, at:at + TH]
                        else:
                            bk, bt = src2
                            ks2 = slice(bk + c * CK, bk + (c + 1) * CK)
                            lhsp = ac_pool.tile([P, CK, TH], BF16, tag="ac")
                            if op == "add":
                                nc.vector.tensor_add(
                                    lhsp[:], hT[:, ks, at:at + TH],
                                    hT[:, ks2, bt:bt + TH])
                            else:
                                nc.vector.tensor_sub(
                                    lhsp[:], hT[:, ks, at:at + TH],
                                    hT[:, ks2, bt:bt + TH])
                        for kk in range(CK):
                            for m in range(2):
                                for fd in range(2):
                                    nc.tensor.matmul(
                                        ps[2 * m + fd][:],
                                        lhsT=lhsp[:, kk, m * P:(m + 1) * P],
                                        rhs=w2cc[:, kk, fd * 512:
                                                 (fd + 1) * 512],
                                        start=(c == 0 and kk == 0
                                               and not has_bias),
                                        stop=(c == NCH - 1 and kk == CK - 1))
                    # drain into Cacc, fused +/-.
                    for quad, mode in DRAIN[i]:
                        qm0, qh0 = QUAD[quad]
                        for m in range(2):
                            for fd in range(2):
                                dst = cacc[:, qm0 + m,
                                           qh0 + fd * 512:qh0 + (fd + 1) * 512]
                                src = ps[2 * m + fd][:]
                                if mode == "cp":
                                    nc.scalar.copy(dst, src)
                                elif mode == "ncp":
                                    nc.scalar.mul(dst, src, -1.0)
                                elif mode == "add":
                                    nc.vector.tensor_add(dst, dst, src)
                                else:
                                    nc.vector.tensor_sub(dst, dst, src)
                    if i in DONE_AFTER:
                        qm0, qh0 = QUAD[DONE_AFTER[i]]
                        for m in range(2):
                            nc.sync.dma_start(
                                out=out_r[:, t, qm0 + m, qh0:qh0 + Hh],
                                in_=cacc[:, qm0 + m, qh0:qh0 + Hh])
                if nxt:
                    xc, slabs = xc_n, slabs_n

    nc.compile()
    return nc


def make_in_maps(x, w1, b1, w2, b2, n_cores=N_CORES):
    """Shard FULL f32 inputs into per-core in_maps (host-side layout prep)."""
    T_core = x.shape[0] // n_cores
    H = x.shape[1]
    F = w1.shape[1]
    FT = F // P
    HT = w2.shape[1] // P
    KH_ = H // P
    TB = 512
    TH = TB // 2
    n_blocks = T_core // TB
    Hh = H // 2
    Fh = F // 2

    # w1 permuted so that Strassen column-pair (j of F-half1, j of F-half2)
    # is a contiguous 256-col slab: w1p[:, j*256 + s*128 + c] = w1[:, s*F/2 + j*128 + c]
    # Then repacked per-slab-contiguous: w1ps[j, p, k, c] = w1p[k*128+p, j*256+c].
    w1p = (
        w1.reshape(H, 2, F // 256, 128).transpose(0, 2, 1, 3).reshape(H, F)
    ).astype(NP_BF16)
    w1ps = np.ascontiguousarray(
        w1p.reshape(KH_, P, F // 256, 256).transpose(2, 1, 0, 3))

    # GEMM2 B-side Strassen pieces of w2 (combined in f32, then bf16),
    # repacked per-chunk-contiguous: [i, kc, p, kk, h].
    B11 = w2[0:Fh, 0:Hh]
    B12 = w2[0:Fh, Hh:H]
    B21 = w2[Fh:F, 0:Hh]
    B22 = w2[Fh:F, Hh:H]
    w2c = np.stack([B11 + B22, B11, B12 - B22, B21 - B11, B22,
                    B11 + B12, B21 + B22]).astype(NP_BF16)
    w2c = np.ascontiguousarray(
        w2c.reshape(7, Fh // P // 4, 4, P, Hh).transpose(0, 1, 3, 2, 4))

    b1t = b1.astype(np.float32).reshape(FT, P).T
    b2t = b2.astype(np.float32).reshape(HT, P).T
    BCOLS = -(-(FT + HT) // P) * P
    bcm = np.zeros((P, BCOLS), dtype=np.float32)
    bcm[:, 0:FT] = b1t
    bcm[:, FT:FT + HT] = b2t

    cst = np.zeros((1, H + P), dtype=NP_BF16)
    cst[0, 0:H] = b2.astype(NP_BF16)
    cst[0, H:H + P] = 1.0

    in_maps = []
    for c in range(n_cores):
        xs = x[c * T_core:(c + 1) * T_core].astype(np.float32)
        # block-contiguous layout: xc[b, p, i*8+kk, tok] = piece_i[kk*128+p, tok]
        xc = np.empty((n_blocks, P, 7 * Hh // P, TH), dtype=NP_BF16)
        for b in range(n_blocks):
            xb = xs[b * TB:(b + 1) * TB]
            A11 = xb[0:TH, 0:Hh]
            A12 = xb[0:TH, Hh:H]
            A21 = xb[TH:TB, 0:Hh]
            A22 = xb[TH:TB, Hh:H]
            combos = (A11 + A22, A21 + A22, A11, A22,
                      A11 + A12, A21 - A11, A12 - A22)
            for i, S in enumerate(combos):
                piece = S.T.astype(NP_BF16).reshape(Hh // P, P, TH)
                xc[b, :, i * (Hh // P):(i + 1) * (Hh // P), :] = \
                    piece.transpose(1, 0, 2)
        in_maps.append({"xc": xc, "w1p": w1ps, "w2c": w2c, "bc": bcm,
                        "cst": cst})
    return in_maps


_CACHE = {}


def _get_nc():
    if "nc" not in _CACHE:
        _CACHE["nc"] = build_mlp(TOKENS // N_CORES, HIDDEN, FFN, TB=512)
    return _CACHE["nc"]


def run(x, w1, b1, w2, b2, trace=False, **kw):
    nc = _get_nc()
    in_maps = make_in_maps(x, w1, b1, w2, b2)
    res = run_bass_kernel_spmd(nc, in_maps, core_ids=list(range(N_CORES)),
                               trace=trace, **kw)
    y = np.concatenate(
        [np.asarray(res.results[i]["out"]) for i in range(N_CORES)], axis=0)
    return np.ascontiguousarray(y.astype(np.float32)), res


def kernel(x, w1, b1, w2, b2):
    x = np.asarray(x, dtype=np.float32)
    w1 = np.asarray(w1, dtype=np.float32)
    b1 = np.asarray(b1, dtype=np.float32)
    w2 = np.asarray(w2, dtype=np.float32)
    b2 = np.asarray(b2, dtype=np.float32)
    y, _ = run(x, w1, b1, w2, b2, trace=False)
    return y
